# revision 27
# baseline (speedup 1.0000x reference)
"""GQA attention kernel for 8 TRN2 NeuronCores.

Sharding: data-parallel over batch (B=2) x tensor-parallel over heads (4-way).
Core i handles batch i//4 and head-shard i%4 (8 query heads = 2 KV groups).
Out-proj is row-sharded; the 4 partial [S,D] outputs per batch are summed on
the host (cheap unshard step), bo added once.

Device kernel (per core, all bf16 matmuls, f32 PSUM), tuned so every matmul
is a uniform (128,128)-tile config (ldweights pipelines behind the previous
matmul; avoids the ~100ns exposed-ld penalty of 64-row configs):
  KT = Wk_sh.T @ kvT          [128, S]
  V  = kvT.T  @ Wv_sh         [S, 128] -> per-group V_aug [S, 64+1] (ones col)
  QT -> zero-padded per-head slabs [128, S]: head h's 64 dims sit in the
  array half matching its KV group's rows of KT, other half zeros, so
  scores use full-128 contraction: S^T = KT_chunk^T @ Qslab.
  per (head, q-chunk 512): scores^T chunks [128 keys, 512 q] -> exp(scale)
  -> causal mask via sliding window of a precomputed [128,1024] 0/1 mask ->
  PV accumulate with ones-row giving softmax sums in row 64 -> normalize via
  reciprocal + e0-matmul broadcast (e0 = [128,128] with row0=1) ->
  OT [128(dims), S] -> out_partial = OT.T @ Wo_sh  [S, D] bf16.
"""

import numpy as np

B, S, D = 2, 2048, 2048
H, G, HD, GS = 32, 8, 64, 4
HPC = 8     # query heads per core
GPC = 2     # kv groups per core
NCORES = 8
SCALE = 0.125  # 1/sqrt(64)

_CACHE = {}


def _build():
    import concourse.bass as bass
    import concourse.tile as tile
    from concourse import bacc, mybir

    f32 = mybir.dt.float32
    f32r = mybir.dt.float32r
    bf16 = mybir.dt.bfloat16
    AF = mybir.ActivationFunctionType
    ALU = mybir.AluOpType

    nc = bacc.Bacc("TRN2", target_bir_lowering=False, debug=False,
                   num_devices=NCORES)

    # pre-chunked host layouts: [128, tch, c, 512] for activations,
    # [128, c, cols] for weights -> all DMAs are contiguous [128, N] copies
    xT_d = nc.declare_dram_parameter("xT", [128, 4 * 16 * 512], bf16,
                                     isOutput=False)
    kvT_d = nc.declare_dram_parameter("kvT", [128, 4 * 16 * 512], bf16,
                                      isOutput=False)
    wq_d = nc.declare_dram_parameter("wq", [128, 16 * 512], bf16,
                                     isOutput=False)
    wk_d = nc.declare_dram_parameter("wk", [128, 16 * 128], bf16,
                                     isOutput=False)
    wv_d = nc.declare_dram_parameter("wv", [128, 16 * 128], bf16,
                                     isOutput=False)
    wo_d = nc.declare_dram_parameter("wo", [128, 4 * 2048], bf16,
                                     isOutput=False)
    bq_d = nc.declare_dram_parameter("bq", [128, 4], f32, isOutput=False)
    bk_d = nc.declare_dram_parameter("bk", [128, 1], f32, isOutput=False)
    bvt_d = nc.declare_dram_parameter("bvt", [128, 2 * 64], f32, isOutput=False)
    m0_d = nc.declare_dram_parameter("m0", [128, 1024], bf16, isOutput=False)
    out_d = nc.declare_dram_parameter("out", [S, D], bf16, isOutput=True)

    with tile.TileContext(nc) as tc:
        with (
            tc.tile_pool(name="persist", bufs=1) as persist,
            tc.tile_pool(name="stream", bufs=3) as stream,
            tc.tile_pool(name="osbp", bufs=3) as osbp,
            tc.tile_pool(name="small", bufs=3) as small,
            tc.tile_pool(name="probs", bufs=6) as probs_pool,
            tc.tile_pool(name="ps_s", bufs=3, space="PSUM") as ps_s,
            tc.tile_pool(name="ps_proj", bufs=2, space="PSUM") as ps_proj,
            tc.tile_pool(name="ps_o", bufs=2, space="PSUM") as ps_o,
            tc.tile_pool(name="ps_b", bufs=1, space="PSUM") as ps_b,
        ):
            # ---- resident tiles ----
            wq_sb = persist.tile([128, 16 * 512], bf16, tag="wq")
            wk_sb = persist.tile([128, 16 * 128], bf16, tag="wk")
            wv_sb = persist.tile([128, 16 * 128], bf16, tag="wv")
            wo_sb = persist.tile([128, 4 * 2048], bf16, tag="wo")
            m0_sb = persist.tile([128, 1024], bf16, tag="m0")
            bq_sb = persist.tile([128, 4], f32, tag="bq")
            bk_sb = persist.tile([128, 1], f32, tag="bk")
            bvt_sb = persist.tile([128, 2 * 64], f32, tag="bvt")
            # e0: row 0 = ones, rest 0 -> broadcast matmul at (128,128) config
            e0_sb = persist.tile([128, 128], bf16, tag="e0")
            # rsb: row 0 carries 1/sums; rows 1-127 zeros (killed by e0)
            rsb_sb = persist.tile([128, 512], bf16, tag="rsb")

            qt_sb = persist.tile([128, 4 * 2048], bf16, tag="qt")
            # two zero-padded KT copies so scores contract over full 128
            # partitions (uniform (128,128) array config for every matmul):
            # ktp0 = [K_g0 | 0], ktp1 = [0 | K_g1] along the partition dim
            ktp0 = persist.tile([128, S], bf16, tag="ktp0")
            ktp1 = persist.tile([128, S], bf16, tag="ktp1")
            ktp = [ktp0, ktp1]
            vaug_sb = persist.tile([128, 2 * 16 * 65], bf16, tag="vaug")
            ot_sb = persist.tile([128, 4 * 2048], bf16, tag="ot")

            # ---- startup memsets (small; vector idle while first DMAs run) ----
            nc.vector.memset(e0_sb[:], 0.0)
            nc.vector.memset(e0_sb[0:1, :], 1.0)
            nc.vector.memset(rsb_sb[:], 0.0)
            nc.vector.memset(ktp[0][64:128, :], 0.0)
            nc.vector.memset(ktp[1][0:64, :], 0.0)
            # all 64 ones-columns of vaug in one strided memset
            nc.vector.memset(
                vaug_sb.rearrange("p (g t j) -> p g t j", g=2, j=65)
                [:, :, :, 64:65], 1.0)

            # wk first, in 4 pieces: K-proj starts after just 128KB lands
            for c4 in range(0, 16, 4):
                nc.sync.dma_start(out=wk_sb[:, c4 * 128:(c4 + 4) * 128],
                                  in_=wk_d[:, c4 * 128:(c4 + 4) * 128])

            # dummy matmuls fill the DMA-bound head: PE busy from ~1us so
            # the DVFS ramp completes before real compute arrives
            warm_ps = ps_b.tile([128, 512], f32, tag="bps", name="warm_ps")
            for _ in range(30):
                nc.tensor.matmul(warm_ps[:], lhsT=e0_sb[:], rhs=rsb_sb[:],
                                 start=True, stop=True)

            # ---- chain emitters ----
            def kv_chain_thunks(tch):
                """K/V projection for kv token chunk tch: DMA + KT + V."""
                th = []
                state = {}

                def dma():
                    kvt = stream.tile([128, 16 * 512], bf16, tag="xs",
                                      name="kvt")
                    base = tch * 16 * 512
                    for c4 in range(0, 16, 4):
                        nc.sync.dma_start(
                            out=kvt[:, c4 * 512:(c4 + 4) * 512],
                            in_=kvT_d[:, base + c4 * 512:
                                      base + (c4 + 4) * 512])
                        if tch == 0 and c4 == 0:
                            nc.sync.dma_start(out=bk_sb[:], in_=bk_d[:, :])
                            nc.sync.dma_start(out=bvt_sb[:], in_=bvt_d[:, :])
                            nc.sync.dma_start(out=wv_sb[:], in_=wv_d[:, :])
                    state["kvt"] = kvt
                    state["kps"] = ps_proj.tile([128, 512], f32, tag="proj",
                                                name="kps")
                th.append(dma)

                def kmm(c):
                    nc.tensor.matmul(
                        state["kps"][:], lhsT=wk_sb[:, c * 128:(c + 1) * 128],
                        rhs=state["kvt"][:, c * 512:(c + 1) * 512],
                        start=(c == 0), stop=(c == 15))
                    if c == 15:
                        nc.vector.tensor_scalar(
                            ktp[0][0:64, tch * 512:(tch + 1) * 512],
                            state["kps"][0:64, :], bk_sb[0:64, 0:1],
                            None, ALU.add)
                        nc.vector.tensor_scalar(
                            ktp[1][64:128, tch * 512:(tch + 1) * 512],
                            state["kps"][64:128, :], bk_sb[64:128, 0:1],
                            None, ALU.add)
                for c in range(16):
                    th.append(lambda c=c: kmm(c))

                def vmm(tt, c):
                    if c == 0:
                        state["vps"] = ps_proj.tile([128, 128], f32, tag="proj",
                                                    name="vps")
                    nc.tensor.matmul(
                        state["vps"][:],
                        lhsT=state["kvt"][:, c * 512 + tt * 128:
                                          c * 512 + (tt + 1) * 128],
                        rhs=wv_sb[:, c * 128:(c + 1) * 128],
                        start=(c == 0), stop=(c == 15))
                    if c == 15:
                        tok = tch * 4 + tt
                        for gl in range(2):
                            base = gl * 1040 + tok * 65
                            nc.vector.tensor_tensor(
                                vaug_sb[:, base:base + 64],
                                state["vps"][:, gl * 64:(gl + 1) * 64],
                                bvt_sb[:, gl * 64:(gl + 1) * 64], ALU.add)
                for tt in range(4):
                    for c in range(0, 16, 4):
                        def v4(tt=tt, c0=c):
                            for c in range(c0, c0 + 4):
                                vmm(tt, c)
                        th.append(v4)
                return th

            def q_chain_thunks(qch, fine=False):
                """Q projection for q chunk qch.

                fine=True (first chunk): wq/xt DMAs go out in 4 pieces so the
                first qmm chain can start as soon as piece 0 lands.
                """
                th = []
                state = {}

                def dma(c4):
                    if c4 == 0:
                        state["xt"] = stream.tile([128, 16 * 512], bf16,
                                                  tag="xs", name="xt")
                        if fine:
                            nc.sync.dma_start(out=bq_sb[:], in_=bq_d[:, :])
                    if fine:
                        nc.sync.dma_start(
                            out=wq_sb[:, c4 * 512:(c4 + 4) * 512],
                            in_=wq_d[:, c4 * 512:(c4 + 4) * 512])
                    base = qch * 16 * 512
                    nc.sync.dma_start(
                        out=state["xt"][:, c4 * 512:(c4 + 4) * 512],
                        in_=xT_d[:, base + c4 * 512:base + (c4 + 4) * 512])

                def qmm(hc, c):
                    if c == 0:
                        state["qps"] = ps_proj.tile([128, 512], f32, tag="proj",
                                                    name="qps")
                    nc.tensor.matmul(
                        state["qps"][:],
                        lhsT=wq_sb[:, c * 512 + hc * 128:c * 512 + (hc + 1) * 128],
                        rhs=state["xt"][:, c * 512:(c + 1) * 512],
                        start=(c == 0), stop=(c == 15))
                    if c == 15:
                        nc.vector.tensor_scalar(
                            qt_sb[:, hc * 2048 + qch * 512:
                                  hc * 2048 + (qch + 1) * 512],
                            state["qps"][:], bq_sb[:, hc:hc + 1], None, ALU.add)

                if fine:
                    for c4 in range(0, 16, 4):
                        th.append(lambda c4=c4: dma(c4))
                else:
                    def dma_all():
                        for c4 in range(0, 16, 4):
                            dma(c4)
                    th.append(dma_all)
                for hc in range(4):
                    for c in range(16):
                        th.append(lambda hc=hc, c=c: qmm(hc, c))
                return th

            def outproj_thunks(jqb):
                """Out-projection for q block jqb (4 q-tiles x 4 col-chunks)."""
                th = []
                state = {}

                def omm(qt_i, cc, c):
                    if c == 0:
                        state["outp"] = ps_proj.tile([128, 512], f32, tag="proj",
                                                     name="outp")
                    nc.tensor.matmul(
                        state["outp"][:],
                        lhsT=ot_sb[:, c * 2048 + qt_i * 128:
                                   c * 2048 + (qt_i + 1) * 128],
                        rhs=wo_sb[:, c * 2048 + cc * 512:c * 2048 + (cc + 1) * 512],
                        start=(c == 0), stop=(c == 3))
                    if c == 3:
                        if cc == 0:
                            state["osb"] = osbp.tile([128, 2048], bf16,
                                                     tag="osb", name="osb")
                        # alternate copy engine: scalar on even cc, vector odd
                        if cc % 2 == 0:
                            nc.scalar.activation(
                                state["osb"][:, cc * 512:(cc + 1) * 512],
                                state["outp"][:], AF.Copy)
                        else:
                            nc.vector.tensor_copy(
                                state["osb"][:, cc * 512:(cc + 1) * 512],
                                state["outp"][:])
                        nc.sync.dma_start(
                            out=out_d[qt_i * 128:(qt_i + 1) * 128,
                                      cc * 512:(cc + 1) * 512],
                            in_=state["osb"][:, cc * 512:(cc + 1) * 512])
                for qt_i in range(jqb * 4, jqb * 4 + 4):
                    for cc in range(4):
                        for c in range(4):
                            th.append(lambda q=qt_i, cc=cc, c=c: omm(q, cc, c))
                return th

            # ---- filler queue machinery ----
            fillers = []
            fpos = [0]

            def pop_filler(n=1):
                while n > 0 and fpos[0] < len(fillers):
                    fillers[fpos[0]]()
                    fpos[0] += 1
                    n -= 1

            def drain_fillers_through(idx):
                while fpos[0] <= idx:
                    fillers[fpos[0]]()
                    fpos[0] += 1

            # ---- attention for one (head, q-chunk), 2-deep QK pipeline ----
            # finalize (normalize) is deferred: emitted a few chunks into the
            # next head so the PE queue never stalls on the vector recip chain
            def attention(h, jq):
                gl = h // 4
                hr = gl * 64
                nkc = 4 * jq + 4
                qbase = (h % 4) * 2048 + jq * 512   # into qt pair-chunk layout
                obase = qbase                        # ot shares that layout
                ops = ps_o.tile([65, 512], f32, tag="ops", name="ops")
                sps_t = {}
                pt_t = {}

                def emit_qk(kci):
                    m = max(0, kci * 128 - jq * 512)
                    sps = ps_s.tile([128, 512], f32, tag="sps", name="sps")
                    # full-128 contraction: the other group's q rows hit the
                    # zero half of the padded KT copy
                    nc.tensor.matmul(
                        sps[:, m:512],
                        lhsT=ktp[gl][:, kci * 128:(kci + 1) * 128],
                        rhs=qt_sb[:, qbase + m:qbase + 512],
                        start=True, stop=True)
                    sps_t[kci] = (sps, m)

                def emit_exp(kci):
                    sps, m = sps_t.pop(kci)
                    pt = probs_pool.tile([128, 512], bf16, tag="pt", name="pt")
                    nc.scalar.activation(pt[:, m:512], sps[:, m:512],
                                         AF.Exp, scale=SCALE)
                    if kci >= 4 * jq:   # diagonal chunk -> mask
                        nc.vector.tensor_tensor(
                            pt[:, m:512], pt[:, m:512],
                            m0_sb[:, 512:1024 - m], ALU.mult)
                    pt_t[kci] = (pt, m)

                def emit_pv(kci):
                    pt, m = pt_t.pop(kci)
                    vbase = gl * 1040 + kci * 65
                    nc.tensor.matmul(
                        ops[:, m:512], lhsT=vaug_sb[:, vbase:vbase + 65],
                        rhs=pt[:, m:512],
                        start=(kci == 0), stop=(kci == nkc - 1))

                emit_qk(0)
                if nkc > 1:
                    emit_qk(1)
                for kci in range(nkc):
                    emit_exp(kci)
                    if kci + 2 < nkc:
                        emit_qk(kci + 2)
                    pop_filler(1)
                    emit_pv(kci)

                def finalize():
                    # sums (row 64 of ops) -> SBUF -> 1/sums -> rsb row0 ->
                    # e0 broadcast matmul (custom-DVE recip can't read PSUM
                    # on hw)
                    rss = small.tile([1, 512], f32, tag="rss", name="rss")
                    nc.vector.tensor_copy(rss[:], ops[64:65, :])
                    rs = small.tile([1, 512], f32, tag="rs", name="rs")
                    nc.vector.reciprocal_approx_fast(rs[:], rss[:])
                    nc.vector.tensor_copy(rsb_sb[0:1, :], rs[:])
                    bps = ps_b.tile([128, 512], f32, tag="bps", name="bps")
                    nc.tensor.matmul(bps[:], lhsT=e0_sb[:], rhs=rsb_sb[:],
                                     start=True, stop=True)
                    bsb = small.tile([64, 512], f32, tag="bsb", name="bsb")
                    nc.scalar.activation(bsb[:], bps[0:64, :], AF.Copy)
                    nc.vector.tensor_tensor(
                        ot_sb[hr:hr + 64, obase:obase + 512],
                        ops[0:64, :], bsb[:], ALU.mult)
                return finalize

            # ---- emission schedule ----
            for t in kv_chain_thunks(0):
                t()
            for t in q_chain_thunks(0, fine=True):
                t()
            nc.sync.dma_start(out=m0_sb[:], in_=m0_d[:, :])
            nc.sync.dma_start(
                out=wo_sb[:, 0:4096], in_=wo_d[:, 0:4096])
            nc.sync.dma_start(
                out=wo_sb[:, 4096:8192], in_=wo_d[:, 4096:8192])

            group_end = {}
            for name, th in [("kv1", kv_chain_thunks(1)),
                             ("kv2", kv_chain_thunks(2)),
                             ("kv3", kv_chain_thunks(3)),
                             ("q1", q_chain_thunks(1)),
                             ("q2", q_chain_thunks(2)),
                             ("q3", q_chain_thunks(3))]:
                fillers.extend(th)
                group_end[name] = len(fillers) - 1

            pending = None
            for jq in range(4):
                if jq >= 1:
                    drain_fillers_through(group_end[f"kv{jq}"])
                    drain_fillers_through(group_end[f"q{jq}"])
                for h in (0, 4, 1, 5, 2, 6, 3, 7):
                    fin = attention(h, jq)
                    if pending is not None:
                        pending()
                    pending = fin
                    pop_filler(2)
                fillers.extend(outproj_thunks(jq))
                group_end[f"op{jq}"] = len(fillers) - 1
                if jq == 3:
                    pending()
                    pending = None
            pop_filler(len(fillers))
    nc.finalize()
    return nc


def _get_nc():
    if "nc" not in _CACHE:
        _CACHE["nc"] = _build()
    return _CACHE["nc"]


def kernel(**inputs):
    out, _ = _run(inputs, trace=False)
    return out


def _chunk_act(a):
    # [D, S] -> [128, tch, c, 512] -> [128, 4*16*512]
    # token chunk tch (512 tokens), dim chunk c (128 dims)
    d, s = a.shape
    a = a.reshape(16, 128, 4, 512)          # [c, p, tch, j]
    a = a.transpose(1, 2, 0, 3)             # [p, tch, c, j]
    return np.ascontiguousarray(a.reshape(128, 4 * 16 * 512))


def _chunk_w(w, cols):
    # [D, cols] -> [128, c, cols] -> [128, 16*cols]
    return np.ascontiguousarray(
        w.reshape(16, 128, cols).transpose(1, 0, 2).reshape(128, 16 * cols))


def _run(inputs, trace=False):
    import ml_dtypes
    from concourse.bass_utils import run_bass_kernel_spmd

    x = np.asarray(inputs["x"], np.float32)
    kv = np.asarray(inputs["kv"], np.float32)
    Wq = np.asarray(inputs["Wq"], np.float32)
    bq = np.asarray(inputs["bq"], np.float32)
    Wk = np.asarray(inputs["Wk"], np.float32)
    bk = np.asarray(inputs["bk"], np.float32)
    Wv = np.asarray(inputs["Wv"], np.float32)
    bv = np.asarray(inputs["bv"], np.float32)
    Wo = np.asarray(inputs["Wo"], np.float32)
    bo = np.asarray(inputs["bo"], np.float32)

    bf = ml_dtypes.bfloat16
    M0 = (np.arange(1024)[None, :] >= (np.arange(128)[:, None] + 512)
          ).astype(bf)

    # head-dim permutation: chunk c = [local head c | local head 4+c]
    hperm = np.concatenate(
        [np.concatenate([np.arange(c * 64, c * 64 + 64),
                         np.arange((4 + c) * 64, (4 + c) * 64 + 64)])
         for c in range(4)])  # [512] permutation of local head dims

    in_maps = []
    for core in range(NCORES):
        b, t = core // 4, core % 4
        bv_sh = bv[t * 128:(t + 1) * 128]
        bvt = np.broadcast_to(bv_sh[None, :], (128, 128)).astype(np.float32)
        wq_sh = Wq[:, t * 512:(t + 1) * 512][:, hperm]
        wo_sh = Wo[t * 512:(t + 1) * 512, :][hperm, :]
        bq_sh = bq[t * 512:(t + 1) * 512][hperm]
        wo_chunked = np.ascontiguousarray(
            wo_sh.reshape(4, 128, 2048).transpose(1, 0, 2).reshape(128, 8192))
        in_maps.append({
            "xT": _chunk_act(x[b].T).astype(bf),
            "kvT": _chunk_act(kv[b].T).astype(bf),
            "wq": _chunk_w(wq_sh, 512).astype(bf),
            "wk": _chunk_w(Wk[:, t * 128:(t + 1) * 128], 128).astype(bf),
            "wv": _chunk_w(Wv[:, t * 128:(t + 1) * 128], 128).astype(bf),
            "wo": wo_chunked.astype(bf),
            "bq": np.ascontiguousarray(bq_sh.reshape(4, 128).T),
            "bk": bk[t * 128:(t + 1) * 128].reshape(128, 1).copy(),
            "bvt": np.ascontiguousarray(bvt),
            "m0": M0,
        })

    nc = _get_nc()
    res = run_bass_kernel_spmd(nc, in_maps, core_ids=list(range(NCORES)),
                               trace=trace)
    parts = [np.asarray(res.results[i]["out"], np.float32)
             for i in range(NCORES)]
    out = np.stack([parts[0] + parts[1] + parts[2] + parts[3],
                    parts[4] + parts[5] + parts[6] + parts[7]])
    out += bo[None, None, :]
    return out.astype(np.float32), res


# revision 34
# speedup vs baseline: 1.0237x; 1.0237x over previous
"""GQA attention kernel for 8 TRN2 NeuronCores.

Sharding: data-parallel over batch (B=2) x tensor-parallel over heads (4-way).
Core i handles batch i//4 and head-shard i%4 (8 query heads = 2 KV groups).
Out-proj is row-sharded; the 4 partial [S,D] outputs per batch are summed on
the host (cheap unshard step), bo added once.

Device kernel (per core, all bf16 matmuls, f32 PSUM), tuned so every matmul
is a uniform (128,128)-tile config (ldweights pipelines behind the previous
matmul; avoids the ~100ns exposed-ld penalty of 64-row configs):
  KT = Wk_sh.T @ kvT          [128, S]
  V  = kvT.T  @ Wv_sh         [S, 128] -> per-group V_aug [S, 64+1] (ones col)
  QT -> zero-padded per-head slabs [128, S]: head h's 64 dims sit in the
  array half matching its KV group's rows of KT, other half zeros, so
  scores use full-128 contraction: S^T = KT_chunk^T @ Qslab.
  per (head, q-chunk 512): scores^T chunks [128 keys, 512 q] -> exp(scale)
  -> causal mask via sliding window of a precomputed [128,1024] 0/1 mask ->
  PV accumulate with ones-row giving softmax sums in row 64 -> normalize via
  reciprocal + e0-matmul broadcast (e0 = [128,128] with row0=1) ->
  OT [128(dims), S] -> out_partial = OT.T @ Wo_sh  [S, D] bf16.
"""

import numpy as np

B, S, D = 2, 2048, 2048
H, G, HD, GS = 32, 8, 64, 4
HPC = 8     # query heads per core
GPC = 2     # kv groups per core
NCORES = 8
SCALE = 0.125  # 1/sqrt(64)

_CACHE = {}


def _build():
    import concourse.bass as bass
    import concourse.tile as tile
    from concourse import bacc, mybir

    f32 = mybir.dt.float32
    f32r = mybir.dt.float32r
    bf16 = mybir.dt.bfloat16
    AF = mybir.ActivationFunctionType
    ALU = mybir.AluOpType

    nc = bacc.Bacc("TRN2", target_bir_lowering=False, debug=False,
                   num_devices=NCORES)

    # pre-chunked host layouts: [128, tch, c, 512] for activations,
    # [128, c, cols] for weights -> all DMAs are contiguous [128, N] copies
    xT_d = nc.declare_dram_parameter("xT", [128, 4 * 16 * 512], bf16,
                                     isOutput=False)
    kvT_d = nc.declare_dram_parameter("kvT", [128, 4 * 16 * 512], bf16,
                                      isOutput=False)
    wq_d = nc.declare_dram_parameter("wq", [128, 16 * 512], bf16,
                                     isOutput=False)
    wk_d = nc.declare_dram_parameter("wk", [128, 16 * 128], bf16,
                                     isOutput=False)
    wv_d = nc.declare_dram_parameter("wv", [128, 16 * 128], bf16,
                                     isOutput=False)
    wo_d = nc.declare_dram_parameter("wo", [128, 4 * 2048], bf16,
                                     isOutput=False)
    bq_d = nc.declare_dram_parameter("bq", [128, 4], f32, isOutput=False)
    bk_d = nc.declare_dram_parameter("bk", [128, 1], f32, isOutput=False)
    bvt_d = nc.declare_dram_parameter("bvt", [128, 2 * 64], f32, isOutput=False)
    m0_d = nc.declare_dram_parameter("m0", [128, 1024], bf16, isOutput=False)
    out_d = nc.declare_dram_parameter("out", [S, D], bf16, isOutput=True)

    with tile.TileContext(nc) as tc:
        with (
            tc.tile_pool(name="persist", bufs=1) as persist,
            tc.tile_pool(name="stream", bufs=3) as stream,
            tc.tile_pool(name="osbp", bufs=3) as osbp,
            tc.tile_pool(name="small", bufs=3) as small,
            tc.tile_pool(name="probs", bufs=6) as probs_pool,
            tc.tile_pool(name="ps_s", bufs=3, space="PSUM") as ps_s,
            tc.tile_pool(name="ps_proj", bufs=2, space="PSUM") as ps_proj,
            tc.tile_pool(name="ps_o", bufs=2, space="PSUM") as ps_o,
            tc.tile_pool(name="ps_b", bufs=1, space="PSUM") as ps_b,
        ):
            # ---- resident tiles ----
            wq_sb = persist.tile([128, 16 * 512], bf16, tag="wq")
            wk_sb = persist.tile([128, 16 * 128], bf16, tag="wk")
            wv_sb = persist.tile([128, 16 * 128], bf16, tag="wv")
            wo_sb = persist.tile([128, 4 * 2048], bf16, tag="wo")
            m0_sb = persist.tile([128, 1024], bf16, tag="m0")
            bq_sb = persist.tile([128, 4], f32, tag="bq")
            bk_sb = persist.tile([128, 1], f32, tag="bk")
            bvt_sb = persist.tile([128, 2 * 64], f32, tag="bvt")
            # e0: row 0 = ones, rest 0 -> broadcast matmul at (128,128) config
            e0_sb = persist.tile([128, 128], bf16, tag="e0")
            # rsb: row 0 carries 1/sums; rows 1-127 zeros (killed by e0)
            rsb_sb = persist.tile([128, 512], bf16, tag="rsb")

            qt_sb = persist.tile([128, 4 * 2048], bf16, tag="qt")
            # two zero-padded KT copies so scores contract over full 128
            # partitions (uniform (128,128) array config for every matmul):
            # ktp0 = [K_g0 | 0], ktp1 = [0 | K_g1] along the partition dim
            ktp0 = persist.tile([128, S], bf16, tag="ktp0")
            ktp1 = persist.tile([128, S], bf16, tag="ktp1")
            ktp = [ktp0, ktp1]
            vaug_sb = persist.tile([128, 2 * 16 * 65], bf16, tag="vaug")
            ot_sb = persist.tile([128, 4 * 2048], bf16, tag="ot")

            # ---- startup memsets (small; vector idle while first DMAs run) ----
            nc.vector.memset(e0_sb[:], 0.0)
            nc.vector.memset(e0_sb[0:1, :], 1.0)
            nc.vector.memset(rsb_sb[:], 0.0)
            nc.vector.memset(ktp[0][64:128, :], 0.0)
            nc.vector.memset(ktp[1][0:64, :], 0.0)
            # all 64 ones-columns of vaug in one strided memset
            nc.vector.memset(
                vaug_sb.rearrange("p (g t j) -> p g t j", g=2, j=65)
                [:, :, :, 64:65], 1.0)

            # wk first, in 4 pieces: K-proj starts after just 128KB lands
            for c4 in range(0, 16, 4):
                nc.sync.dma_start(out=wk_sb[:, c4 * 128:(c4 + 4) * 128],
                                  in_=wk_d[:, c4 * 128:(c4 + 4) * 128])

            # dummy matmuls fill the DMA-bound head: PE busy from ~1us so
            # the DVFS ramp completes before real compute arrives
            warm_ps = ps_b.tile([128, 512], f32, tag="bps", name="warm_ps")
            for _ in range(30):
                nc.tensor.matmul(warm_ps[:], lhsT=e0_sb[:], rhs=rsb_sb[:],
                                 start=True, stop=True)

            # ---- chain emitters ----
            def kv_chain_thunks(tch):
                """K/V projection for kv token chunk tch: DMA + KT + V."""
                th = []
                state = {}

                def dma():
                    kvt = stream.tile([128, 16 * 512], bf16, tag="xs",
                                      name="kvt")
                    base = tch * 16 * 512
                    for c4 in range(0, 16, 4):
                        nc.sync.dma_start(
                            out=kvt[:, c4 * 512:(c4 + 4) * 512],
                            in_=kvT_d[:, base + c4 * 512:
                                      base + (c4 + 4) * 512])
                        if tch == 0 and c4 == 0:
                            nc.sync.dma_start(out=bk_sb[:], in_=bk_d[:, :])
                            nc.sync.dma_start(out=bvt_sb[:], in_=bvt_d[:, :])
                            nc.sync.dma_start(out=wv_sb[:], in_=wv_d[:, :])
                    state["kvt"] = kvt
                    state["kps"] = ps_proj.tile([128, 512], f32, tag="proj",
                                                name="kps")
                th.append(dma)

                def kmm(c):
                    nc.tensor.matmul(
                        state["kps"][:], lhsT=wk_sb[:, c * 128:(c + 1) * 128],
                        rhs=state["kvt"][:, c * 512:(c + 1) * 512],
                        start=(c == 0), stop=(c == 15))
                    if c == 15:
                        # scalar engine: per-partition bias via activation
                        nc.scalar.add(
                            ktp[0][0:64, tch * 512:(tch + 1) * 512],
                            state["kps"][0:64, :], bk_sb[0:64, 0:1])
                        nc.scalar.add(
                            ktp[1][64:128, tch * 512:(tch + 1) * 512],
                            state["kps"][64:128, :], bk_sb[64:128, 0:1])
                for c in range(16):
                    th.append(lambda c=c: kmm(c))

                def vmm(tt, c):
                    if c == 0:
                        state["vps"] = ps_proj.tile([128, 128], f32, tag="proj",
                                                    name="vps")
                    nc.tensor.matmul(
                        state["vps"][:],
                        lhsT=state["kvt"][:, c * 512 + tt * 128:
                                          c * 512 + (tt + 1) * 128],
                        rhs=wv_sb[:, c * 128:(c + 1) * 128],
                        start=(c == 0), stop=(c == 15))
                    if c == 15:
                        tok = tch * 4 + tt
                        for gl in range(2):
                            base = gl * 1040 + tok * 65
                            nc.vector.tensor_tensor(
                                vaug_sb[:, base:base + 64],
                                state["vps"][:, gl * 64:(gl + 1) * 64],
                                bvt_sb[:, gl * 64:(gl + 1) * 64], ALU.add)
                for tt in range(4):
                    for c in range(0, 16, 4):
                        def v4(tt=tt, c0=c):
                            for c in range(c0, c0 + 4):
                                vmm(tt, c)
                        th.append(v4)
                return th

            def q_chain_thunks(qch, fine=False):
                """Q projection for q chunk qch.

                fine=True (first chunk): wq/xt DMAs go out in 4 pieces so the
                first qmm chain can start as soon as piece 0 lands.
                """
                th = []
                state = {}

                def dma(c4):
                    if c4 == 0:
                        state["xt"] = stream.tile([128, 16 * 512], bf16,
                                                  tag="xs", name="xt")
                        if fine:
                            nc.sync.dma_start(out=bq_sb[:], in_=bq_d[:, :])
                    if fine:
                        nc.sync.dma_start(
                            out=wq_sb[:, c4 * 512:(c4 + 4) * 512],
                            in_=wq_d[:, c4 * 512:(c4 + 4) * 512])
                    base = qch * 16 * 512
                    nc.sync.dma_start(
                        out=state["xt"][:, c4 * 512:(c4 + 4) * 512],
                        in_=xT_d[:, base + c4 * 512:base + (c4 + 4) * 512])

                def qmm(hc, c):
                    if c == 0:
                        state["qps"] = ps_proj.tile([128, 512], f32, tag="proj",
                                                    name="qps")
                    nc.tensor.matmul(
                        state["qps"][:],
                        lhsT=wq_sb[:, c * 512 + hc * 128:c * 512 + (hc + 1) * 128],
                        rhs=state["xt"][:, c * 512:(c + 1) * 512],
                        start=(c == 0), stop=(c == 15))
                    if c == 15:
                        nc.scalar.add(
                            qt_sb[:, hc * 2048 + qch * 512:
                                  hc * 2048 + (qch + 1) * 512],
                            state["qps"][:], bq_sb[:, hc:hc + 1])

                if fine:
                    for c4 in range(0, 16, 4):
                        th.append(lambda c4=c4: dma(c4))
                else:
                    def dma_all():
                        for c4 in range(0, 16, 4):
                            dma(c4)
                    th.append(dma_all)
                for hc in range(4):
                    for c in range(16):
                        th.append(lambda hc=hc, c=c: qmm(hc, c))
                return th

            def outproj_thunks(jqb):
                """Out-projection for q block jqb (4 q-tiles x 4 col-chunks).

                Emission is software-pipelined: each (qt,cc) chain's c0-c2
                accumulations go out before the previous chain's closing c3,
                so the c3 matmuls (which depend on the last heads' normalize)
                always have independent work queued ahead of them.
                """
                th = []
                state = {}

                def omm(qt_i, cc, c):
                    key = ("outp", qt_i, cc)
                    if c == 0:
                        state[key] = ps_proj.tile([128, 512], f32, tag="proj",
                                                  name="outp")
                    nc.tensor.matmul(
                        state[key][:],
                        lhsT=ot_sb[:, c * 2048 + qt_i * 128:
                                   c * 2048 + (qt_i + 1) * 128],
                        rhs=wo_sb[:, c * 2048 + cc * 512:c * 2048 + (cc + 1) * 512],
                        start=(c == 0), stop=(c == 3))
                    if c == 3:
                        outp = state.pop(key)
                        if cc == 0:
                            state["osb"] = osbp.tile([128, 2048], bf16,
                                                     tag="osb", name="osb")
                        # early blocks pop while scalar (exp) is light and
                        # vector is slammed; late blocks the reverse
                        if jqb <= 1:
                            nc.scalar.activation(
                                state["osb"][:, cc * 512:(cc + 1) * 512],
                                outp[:], AF.Copy)
                        else:
                            nc.vector.tensor_copy(
                                state["osb"][:, cc * 512:(cc + 1) * 512],
                                outp[:])
                        nc.sync.dma_start(
                            out=out_d[qt_i * 128:(qt_i + 1) * 128,
                                      cc * 512:(cc + 1) * 512],
                            in_=state["osb"][:, cc * 512:(cc + 1) * 512])
                chains = [(qt, cc) for qt in range(jqb * 4, jqb * 4 + 4)
                          for cc in range(4)]
                pend = []
                for q, cc in chains:
                    for c in range(3):
                        th.append(lambda q=q, cc=cc, c=c: omm(q, cc, c))
                    pend.append((q, cc))
                    if len(pend) == 2:
                        q0, cc0 = pend.pop(0)
                        th.append(lambda q=q0, cc=cc0: omm(q, cc, 3))
                while pend:
                    q0, cc0 = pend.pop(0)
                    th.append(lambda q=q0, cc=cc0: omm(q, cc, 3))
                return th

            # ---- filler queue machinery ----
            fillers = []
            fpos = [0]

            def pop_filler(n=1):
                while n > 0 and fpos[0] < len(fillers):
                    fillers[fpos[0]]()
                    fpos[0] += 1
                    n -= 1

            def drain_fillers_through(idx):
                while fpos[0] <= idx:
                    fillers[fpos[0]]()
                    fpos[0] += 1

            # ---- attention for one (head, q-chunk), 2-deep QK pipeline ----
            # finalize (normalize) is deferred: emitted a few chunks into the
            # next head so the PE queue never stalls on the vector recip chain
            def attention(h, jq):
                gl = h // 4
                hr = gl * 64
                nkc = 4 * jq + 4
                qbase = (h % 4) * 2048 + jq * 512   # into qt pair-chunk layout
                obase = qbase                        # ot shares that layout
                ops = ps_o.tile([65, 512], f32, tag="ops", name="ops")
                sps_t = {}
                pt_t = {}

                def emit_qk(kci):
                    m = max(0, kci * 128 - jq * 512)
                    sps = ps_s.tile([128, 512], f32, tag="sps", name="sps")
                    # full-128 contraction: the other group's q rows hit the
                    # zero half of the padded KT copy
                    nc.tensor.matmul(
                        sps[:, m:512],
                        lhsT=ktp[gl][:, kci * 128:(kci + 1) * 128],
                        rhs=qt_sb[:, qbase + m:qbase + 512],
                        start=True, stop=True)
                    sps_t[kci] = (sps, m)

                def emit_exp(kci):
                    sps, m = sps_t.pop(kci)
                    pt = probs_pool.tile([128, 512], bf16, tag="pt", name="pt")
                    nc.scalar.activation(pt[:, m:512], sps[:, m:512],
                                         AF.Exp, scale=SCALE)
                    if kci >= 4 * jq:   # diagonal chunk -> mask
                        nc.vector.tensor_tensor(
                            pt[:, m:512], pt[:, m:512],
                            m0_sb[:, 512:1024 - m], ALU.mult)
                    pt_t[kci] = (pt, m)

                def emit_pv(kci):
                    pt, m = pt_t.pop(kci)
                    vbase = gl * 1040 + kci * 65
                    nc.tensor.matmul(
                        ops[:, m:512], lhsT=vaug_sb[:, vbase:vbase + 65],
                        rhs=pt[:, m:512],
                        start=(kci == 0), stop=(kci == nkc - 1))

                emit_qk(0)
                if nkc > 1:
                    emit_qk(1)
                for kci in range(nkc):
                    emit_exp(kci)
                    if kci + 2 < nkc:
                        emit_qk(kci + 2)
                    pop_filler(1)
                    emit_pv(kci)

                def finalize():
                    # sums (row 64 of ops) -> SBUF -> 1/sums -> rsb row0 ->
                    # e0 broadcast matmul (custom-DVE recip can't read PSUM
                    # on hw)
                    rss = small.tile([1, 512], f32, tag="rss", name="rss")
                    nc.vector.tensor_copy(rss[:], ops[64:65, :])
                    rs = small.tile([1, 512], f32, tag="rs", name="rs")
                    nc.vector.reciprocal_approx_fast(rs[:], rss[:])
                    nc.vector.tensor_copy(rsb_sb[0:1, :], rs[:])
                    bps = ps_b.tile([128, 512], f32, tag="bps", name="bps")
                    nc.tensor.matmul(bps[:], lhsT=e0_sb[:], rhs=rsb_sb[:],
                                     start=True, stop=True)
                    bsb = small.tile([64, 512], f32, tag="bsb", name="bsb")
                    if jq <= 1:
                        nc.scalar.activation(bsb[:], bps[0:64, :], AF.Copy)
                    else:
                        nc.vector.tensor_copy(bsb[:], bps[0:64, :])
                    nc.vector.tensor_tensor(
                        ot_sb[hr:hr + 64, obase:obase + 512],
                        ops[0:64, :], bsb[:], ALU.mult)
                return finalize

            # ---- emission schedule ----
            for t in kv_chain_thunks(0):
                t()
            for t in q_chain_thunks(0, fine=True):
                t()
            nc.sync.dma_start(out=m0_sb[:], in_=m0_d[:, :])
            nc.sync.dma_start(
                out=wo_sb[:, 0:4096], in_=wo_d[:, 0:4096])
            nc.sync.dma_start(
                out=wo_sb[:, 4096:8192], in_=wo_d[:, 4096:8192])

            group_end = {}
            for name, th in [("kv1", kv_chain_thunks(1)),
                             ("kv2", kv_chain_thunks(2)),
                             ("kv3", kv_chain_thunks(3)),
                             ("q1", q_chain_thunks(1)),
                             ("q2", q_chain_thunks(2)),
                             ("q3", q_chain_thunks(3))]:
                fillers.extend(th)
                group_end[name] = len(fillers) - 1

            pending = None
            for jq in range(4):
                if jq >= 1:
                    drain_fillers_through(group_end[f"kv{jq}"])
                    drain_fillers_through(group_end[f"q{jq}"])
                for h in (0, 4, 1, 5, 2, 6, 3, 7):
                    fin = attention(h, jq)
                    if pending is not None:
                        pending()
                    pending = fin
                    pop_filler(2)
                fillers.extend(outproj_thunks(jq))
                group_end[f"op{jq}"] = len(fillers) - 1
                if jq == 3:
                    # queue independent out-proj work (two chains' c0-c2)
                    # ahead of the last head's normalize chain
                    pop_filler(6)
                    pending()
                    pending = None
            pop_filler(len(fillers))
    nc.finalize()
    return nc


def _get_nc():
    if "nc" not in _CACHE:
        _CACHE["nc"] = _build()
    return _CACHE["nc"]


def kernel(**inputs):
    out, _ = _run(inputs, trace=False)
    return out


def _chunk_act(a):
    # [D, S] -> [128, tch, c, 512] -> [128, 4*16*512]
    # token chunk tch (512 tokens), dim chunk c (128 dims)
    d, s = a.shape
    a = a.reshape(16, 128, 4, 512)          # [c, p, tch, j]
    a = a.transpose(1, 2, 0, 3)             # [p, tch, c, j]
    return np.ascontiguousarray(a.reshape(128, 4 * 16 * 512))


def _chunk_w(w, cols):
    # [D, cols] -> [128, c, cols] -> [128, 16*cols]
    return np.ascontiguousarray(
        w.reshape(16, 128, cols).transpose(1, 0, 2).reshape(128, 16 * cols))


def _run(inputs, trace=False):
    import ml_dtypes
    from concourse.bass_utils import run_bass_kernel_spmd

    x = np.asarray(inputs["x"], np.float32)
    kv = np.asarray(inputs["kv"], np.float32)
    Wq = np.asarray(inputs["Wq"], np.float32)
    bq = np.asarray(inputs["bq"], np.float32)
    Wk = np.asarray(inputs["Wk"], np.float32)
    bk = np.asarray(inputs["bk"], np.float32)
    Wv = np.asarray(inputs["Wv"], np.float32)
    bv = np.asarray(inputs["bv"], np.float32)
    Wo = np.asarray(inputs["Wo"], np.float32)
    bo = np.asarray(inputs["bo"], np.float32)

    bf = ml_dtypes.bfloat16
    M0 = (np.arange(1024)[None, :] >= (np.arange(128)[:, None] + 512)
          ).astype(bf)

    # head-dim permutation: chunk c = [local head c | local head 4+c]
    hperm = np.concatenate(
        [np.concatenate([np.arange(c * 64, c * 64 + 64),
                         np.arange((4 + c) * 64, (4 + c) * 64 + 64)])
         for c in range(4)])  # [512] permutation of local head dims

    in_maps = []
    for core in range(NCORES):
        b, t = core // 4, core % 4
        bv_sh = bv[t * 128:(t + 1) * 128]
        bvt = np.broadcast_to(bv_sh[None, :], (128, 128)).astype(np.float32)
        wq_sh = Wq[:, t * 512:(t + 1) * 512][:, hperm]
        wo_sh = Wo[t * 512:(t + 1) * 512, :][hperm, :]
        bq_sh = bq[t * 512:(t + 1) * 512][hperm]
        wo_chunked = np.ascontiguousarray(
            wo_sh.reshape(4, 128, 2048).transpose(1, 0, 2).reshape(128, 8192))
        in_maps.append({
            "xT": _chunk_act(x[b].T).astype(bf),
            "kvT": _chunk_act(kv[b].T).astype(bf),
            "wq": _chunk_w(wq_sh, 512).astype(bf),
            "wk": _chunk_w(Wk[:, t * 128:(t + 1) * 128], 128).astype(bf),
            "wv": _chunk_w(Wv[:, t * 128:(t + 1) * 128], 128).astype(bf),
            "wo": wo_chunked.astype(bf),
            "bq": np.ascontiguousarray(bq_sh.reshape(4, 128).T),
            "bk": bk[t * 128:(t + 1) * 128].reshape(128, 1).copy(),
            "bvt": np.ascontiguousarray(bvt),
            "m0": M0,
        })

    nc = _get_nc()
    res = run_bass_kernel_spmd(nc, in_maps, core_ids=list(range(NCORES)),
                               trace=trace)
    parts = [np.asarray(res.results[i]["out"], np.float32)
             for i in range(NCORES)]
    out = np.stack([parts[0] + parts[1] + parts[2] + parts[3],
                    parts[4] + parts[5] + parts[6] + parts[7]])
    out += bo[None, None, :]
    return out.astype(np.float32), res
